# revision 1
# baseline (speedup 1.0000x reference)
"""Multi-head attention (RoPE, causal) Trainium2 kernel.

Problem: B=2, L=2048, D=2048, H=16, dh=128, fp32.
Sharding: 8 cores = 2 batches x 4 head-groups (4 heads/core).
Each core computes QKV projections for its heads, RoPE, causal
attention, and a partial output projection (its heads' rows of Wo);
the host sums the 4 partials per batch.

Layout strategy (no on-device transposes of activations):
 - host uploads xT = x[b].T; Q/K produced transposed [d, l]; V natural
   [l, d]; scores computed transposed ST[k, q]; exp(ST) in [k, q] is
   directly the moving operand of the AV matmul with V as stationary,
   giving UT[d, q] - exactly the Wo-matmul stationary layout.
 - softmax without max subtraction (scores bounded); row sums via a
   ones-vector matmul of the DVE-accumulated column-sum tile; the 1/r
   normalization enters as a PE ones-outer-product broadcast multiplied
   into UT during PSUM eviction.
 - RoPE rotate-half as a signed-permutation matmul (R^T stationary)
   after a host-side even/odd deinterleave of the Wq/Wk rows.
All matmuls run in float32r (fast fp32, ~1e-4 relative error).
Weights/x are uploaded in pre-tiled layouts so every DMA is a large
contiguous(ish) transfer (descriptor-gen on the HWDGE queue is the
scarce resource, ~600ns fixed per dma_start).
"""
import sys
import numpy as np

sys.path.insert(0, '/opt/trn_rl_repo')

import concourse.bass as bass  # noqa: E402,F401
import concourse.mybir as mybir  # noqa: E402
import concourse.tile as tile  # noqa: E402
from concourse import bacc  # noqa: E402
from concourse import library_config  # noqa: E402
from concourse.bass_utils import run_bass_kernel_spmd  # noqa: E402

B, L, D = 2, 2048, 2048
H, DH = 16, 128
HG = 4           # heads per core
G = H // HG      # head groups (cores per batch)
NCORES = 8
CHUNK = 512      # l-chunk
NCH = L // CHUNK          # 4 chunks
KT = D // 128             # 16 k-tiles over D
LT = L // 128             # 16 l-tiles
ROPE_BASE = 10000.0

f32 = mybir.dt.float32
f32r = mybir.dt.float32r
bf16 = mybir.dt.bfloat16

_built = None
PHASES = []


def _build():
    nc = bacc.Bacc()

    xt_d = nc.declare_dram_parameter("xt", [D, L], f32r, isOutput=False)
    # wq/wk: [m][p][kt*128+f] = W^T[kt*128+p, m*128+f]
    wq_d = nc.declare_dram_parameter("wq", [HG, 128, KT * 128], f32r, isOutput=False)
    wk_d = nc.declare_dram_parameter("wk", [HG, 128, KT * 128], f32r, isOutput=False)
    # wv: [half][p][kt*256+f] = Wv^T[kt*128+p, half*256+f]
    wv_d = nc.declare_dram_parameter("wv", [2, 128, KT * 256], f32r, isOutput=False)
    wo_d = nc.declare_dram_parameter("wo", [HG, 128, D], f32r, isOutput=False)
    cos_d = nc.declare_dram_parameter("cosT", [128, L], f32r, isOutput=False)
    sin_d = nc.declare_dram_parameter("sinT", [128, L], f32r, isOutput=False)
    mask_d = nc.declare_dram_parameter("masks", [4, 128, CHUNK], bf16, isOutput=False)
    permr_d = nc.declare_dram_parameter("permr", [128, 128], f32r, isOutput=False)
    ones_c_d = nc.declare_dram_parameter("ones_c", [128, 1], f32r, isOutput=False)
    ones_r_d = nc.declare_dram_parameter("ones_r", [1, 128], f32r, isOutput=False)

    out_d = nc.declare_dram_parameter("out", [L, D], f32, isOutput=True)

    with tile.TileContext(nc) as tc:
        with (
            tc.tile_pool(name="const", bufs=1) as const,
            tc.tile_pool(name="persist", bufs=1) as persist,
            tc.tile_pool(name="cspool", bufs=4) as cspool,    # cos/sin chunks
            tc.tile_pool(name="xs", bufs=5) as xs,            # x-chunk quarters
            tc.tile_pool(name="wqk", bufs=2) as wqkp,         # per-m weight strips
            tc.tile_pool(name="wvp", bufs=1) as wvp,          # V-weight half
            tc.tile_pool(name="wop", bufs=6) as wop,          # Wo tiles
            tc.tile_pool(name="chact", bufs=4) as chact,      # per-chunk qt/at
            tc.tile_pool(name="tmp32", bufs=6) as tmp32,      # f32 transients
            tc.tile_pool(name="tmpr", bufs=5) as tmpr,        # f32r transients
            tc.tile_pool(name="small", bufs=2) as small,      # [1,512] tiles
            tc.tile_pool(name="pacc", bufs=3, space="PSUM") as pacc,
            tc.tile_pool(name="ptmp", bufs=3, space="PSUM") as ptmp,
            tc.tile_pool(name="prb", bufs=1, space="PSUM") as prb,
            tc.tile_pool(name="pwo", bufs=1, space="PSUM") as pwo,
        ):
            # ---- constants ----
            nc.gpsimd.load_library(library_config.attn)
            masks_t = const.tile([128, 4, CHUNK], bf16)
            permr_t = const.tile([128, 128], f32r)
            ones_c = const.tile([128, 1], f32r)
            ones_r = const.tile([1, 128], f32r)
            nc.sync.dma_start(out=masks_t[:], in_=mask_d[:].rearrange("j p n -> p j n"))
            nc.sync.dma_start(out=permr_t[:], in_=permr_d[:])
            nc.sync.dma_start(out=ones_c[:], in_=ones_c_d[:])
            nc.sync.dma_start(out=ones_r[:], in_=ones_r_d[:])

            # ---- persistent activations (full history) ----
            kt_t = [persist.tile([128, L], f32r, name=f"ktt{h}") for h in range(HG)]
            v_t = [persist.tile([128, HG * 128], f32r, name=f"vt{lt}")
                   for lt in range(LT)]

            for c in range(NCH):
                PHASES.append((f"c{c}_load", int(nc.next_id())))
                cs = slice(c * CHUNK, (c + 1) * CHUNK)
                # ---------- streamed inputs for chunk c ----------
                cos_c = cspool.tile([128, CHUNK], f32r, tag="cs")
                nc.sync.dma_start(out=cos_c[:], in_=cos_d[:, cs])
                sin_c = cspool.tile([128, CHUNK], f32r, tag="cs")
                nc.sync.dma_start(out=sin_c[:], in_=sin_d[:, cs])

                xc = []   # xc[q] = [128, 4, CHUNK]; k-tile kt -> xc[kt//4][:, kt%4, :]
                for q in range(4):
                    xq = xs.tile([128, 4, CHUNK], f32r, tag="xc")
                    nc.sync.dma_start(
                        out=xq[:],
                        in_=xt_d[q * 512:(q + 1) * 512, cs].rearrange(
                            "(kt p) n -> p kt n", p=128))
                    xc.append(xq)

                def xtile(kt):
                    return xc[kt // 4][:, kt % 4, :]

                PHASES.append((f"c{c}_qk", int(nc.next_id())))
                # ---------- Q/K projections + RoPE ----------
                qt_c = [chact.tile([128, CHUNK], f32r, tag="qtc", name=f"qtc{h}")
                        for h in range(HG)]
                for (w_d_, isq) in ((wq_d, True), (wk_d, False)):
                    for m in range(HG):
                        wm = wqkp.tile([128, KT * 128], f32r, tag="wqk")
                        nc.sync.dma_start(out=wm[:], in_=w_d_[m])
                        ps = pacc.tile([128, CHUNK], f32, tag="acc")
                        for kt in range(KT):
                            nc.tensor.matmul(ps[:], wm[:, kt * 128:(kt + 1) * 128],
                                             xtile(kt),
                                             start=(kt == 0), stop=(kt == KT - 1))
                        # RoPE: out = raw*cos + (R @ raw)*sin
                        qraw = tmpr.tile([128, CHUNK], f32r, tag="tmpr")
                        nc.scalar.copy(qraw[:], ps[:])
                        rot = ptmp.tile([128, CHUNK], f32, tag="tmp")
                        nc.tensor.matmul(rot[:], permr_t[:], qraw[:],
                                         start=True, stop=True)
                        t1 = tmp32.tile([128, CHUNK], f32, tag="tmp32")
                        nc.vector.tensor_tensor(out=t1[:], in0=qraw[:].bitcast(f32),
                                                in1=cos_c[:].bitcast(f32),
                                                op=mybir.AluOpType.mult)
                        t2 = tmp32.tile([128, CHUNK], f32, tag="tmp32")
                        nc.vector.tensor_tensor(out=t2[:], in0=rot[:],
                                                in1=sin_c[:].bitcast(f32),
                                                op=mybir.AluOpType.mult)
                        dst = qt_c[m] if isq else kt_t[m]
                        dst_ap = dst[:] if isq else dst[:, cs]
                        nc.vector.tensor_tensor(out=dst_ap, in0=t1[:], in1=t2[:],
                                                op=mybir.AluOpType.add)

                PHASES.append((f"c{c}_v", int(nc.next_id())))
                # ---------- V projection (d in halves) ----------
                for dh2 in range(2):
                    wvh = wvp.tile([128, KT * 256], f32r, tag="wvh")
                    nc.sync.dma_start(out=wvh[:], in_=wv_d[dh2])
                    for sl in range(CHUNK // 128):
                        lt = c * (CHUNK // 128) + sl
                        ps = pacc.tile([128, 256], f32, tag="acc")
                        for kt in range(KT):
                            nc.tensor.matmul(
                                ps[:], xtile(kt)[:, sl * 128:(sl + 1) * 128],
                                wvh[:, kt * 256:(kt + 1) * 256],
                                start=(kt == 0), stop=(kt == KT - 1))
                        nc.scalar.copy(v_t[lt][:, dh2 * 256:(dh2 + 1) * 256], ps[:])

                PHASES.append((f"c{c}_attn", int(nc.next_id())))
                # ---------- attention for q-chunk c ----------
                nkt = (c + 1) * (CHUNK // 128)   # causal: k-tiles 0..nkt-1
                at_c = [chact.tile([128, CHUNK], f32r, tag="atc", name=f"atc{h}")
                        for h in range(HG)]
                for h in range(HG):
                    ut = pacc.tile([128, CHUNK], f32, tag="acc")
                    rsum = prb.tile([1, CHUNK], f32, tag="rb")
                    for kt in range(nkt):
                        st = ptmp.tile([128, CHUNK], f32, tag="tmp")
                        nc.tensor.matmul(st[:], kt_t[h][:, kt * 128:(kt + 1) * 128],
                                         qt_c[h][:], start=True, stop=True)
                        et = tmpr.tile([128, CHUNK], f32r, tag="tmpr")
                        diag_j = kt - (nkt - 4)
                        if diag_j >= 0:
                            eraw = tmp32.tile([128, CHUNK], f32, tag="tmp32")
                            nc.scalar.activation(eraw[:], st[:],
                                                 mybir.ActivationFunctionType.Exp)
                            nc.vector.tensor_tensor(
                                out=et[:], in0=eraw[:],
                                in1=masks_t[:, diag_j, :],
                                op=mybir.AluOpType.mult)
                        else:
                            nc.scalar.activation(et[:], st[:],
                                                 mybir.ActivationFunctionType.Exp)
                        nc.tensor.matmul(ut[:], v_t[kt][:, h * 128:(h + 1) * 128],
                                         et[:], start=(kt == 0), stop=(kt == nkt - 1))
                        nc.tensor.matmul(rsum[:], ones_c[:], et[:],
                                         start=(kt == 0), stop=(kt == nkt - 1))
                    recip = small.tile([1, CHUNK], f32, tag="recip")
                    nc.vector.reciprocal_approx_fast(out=recip[:], in_=rsum[:])
                    bc_sb = tmp32.tile([128, CHUNK], f32, tag="tmp32")
                    nc.gpsimd.partition_broadcast(bc_sb[:], recip[:])
                    nc.vector.tensor_tensor(out=at_c[h][:], in0=ut[:],
                                            in1=bc_sb[:], op=mybir.AluOpType.mult)

                # ---------- output projection for chunk c ----------
                for ot in range(4):
                    wo_tiles = []
                    for h in range(HG):
                        wot = wop.tile([128, 512], f32r, tag="wo")
                        nc.sync.dma_start(
                            out=wot[:], in_=wo_d[h][:, ot * 512:(ot + 1) * 512])
                        wo_tiles.append(wot)
                    for sl in range(CHUNK // 128):
                        mt = c * (CHUNK // 128) + sl
                        ops = pwo.tile([128, 512], f32, tag="wops")
                        for h in range(HG):
                            nc.tensor.matmul(
                                ops[:], at_c[h][:, sl * 128:(sl + 1) * 128],
                                wo_tiles[h][:],
                                start=(h == 0), stop=(h == HG - 1))
                        osb = tmp32.tile([128, 512], f32, tag="tmp32")
                        nc.vector.tensor_copy(out=osb[:], in_=ops[:])
                        nc.scalar.dma_start(
                            out=out_d[mt * 128:(mt + 1) * 128, ot * 512:(ot + 1) * 512],
                            in_=osb[:])

    nc.finalize()
    return nc


def _get_nc():
    global _built
    if _built is None:
        _built = _build()
    return _built


def _host_prep(x, positions, Wq, Wk, Wv, Wo):
    """Build per-core input maps."""
    import ml_dtypes
    x = np.asarray(x, np.float32)
    positions = np.asarray(positions)
    Wq = np.asarray(Wq, np.float32)
    Wk = np.asarray(Wk, np.float32)
    Wv = np.asarray(Wv, np.float32)
    Wo = np.asarray(Wo, np.float32)

    scale = np.float32(1.0 / np.sqrt(DH))
    perm = np.concatenate([np.arange(0, DH, 2), np.arange(1, DH, 2)])  # deinterleave

    Wq_p = (Wq * scale).reshape(H, DH, D)[:, perm, :]   # [H, dh, D]
    Wk_p = Wk.reshape(H, DH, D)[:, perm, :]

    # RoPE tables per batch (deinterleaved: first 64 = even dims, last 64 = odd)
    inv_freq = 1.0 / (ROPE_BASE ** (np.arange(0, DH, 2, dtype=np.float32) / DH))
    cosT = np.empty((B, 128, L), np.float32)
    sinT = np.empty((B, 128, L), np.float32)
    for b in range(B):
        freqs = positions[b].astype(np.float32)[:, None] * inv_freq[None, :]  # [L, 64]
        cb = np.cos(freqs).T.astype(np.float32)  # [64, L]
        sb = np.sin(freqs).T.astype(np.float32)
        cosT[b] = np.concatenate([cb, cb], axis=0)
        sinT[b] = np.concatenate([sb, sb], axis=0)

    # rotate-half signed permutation (in deinterleaved space), uploaded as R.T
    R = np.zeros((128, 128), np.float32)
    for i in range(64):
        R[i, i + 64] = -1.0
        R[i + 64, i] = 1.0
    permr = R.T.copy()

    # causal masks for diagonal blocks (0/1, exact in bf16)
    masks = np.zeros((4, 128, CHUNK), np.float32)
    for j in range(4):
        kk = j * 128 + np.arange(128)[:, None]
        qq = np.arange(CHUNK)[None, :]
        masks[j] = (kk <= qq).astype(np.float32)
    masks = masks.astype(ml_dtypes.bfloat16)

    ones_c = np.ones((128, 1), np.float32)
    ones_r = np.ones((1, 128), np.float32)

    in_maps = []
    for core in range(NCORES):
        b, g = divmod(core, G)
        hs = slice(g * HG, (g + 1) * HG)
        # W^T for this core's heads: [D, HG*dh]
        wqT = Wq_p[hs].reshape(HG * DH, D).T          # [D, 512]
        wkT = Wk_p[hs].reshape(HG * DH, D).T
        wvT = Wv.reshape(H, DH, D)[hs].reshape(HG * DH, D).T
        # [m][p][kt*128+f] layout
        wq_c = np.ascontiguousarray(
            wqT.reshape(KT, 128, HG, DH).transpose(2, 1, 0, 3).reshape(
                HG, 128, KT * DH))
        wk_c = np.ascontiguousarray(
            wkT.reshape(KT, 128, HG, DH).transpose(2, 1, 0, 3).reshape(
                HG, 128, KT * DH))
        # [half][p][kt*256+f]
        wv_c = np.ascontiguousarray(
            wvT.reshape(KT, 128, 2, 256).transpose(2, 1, 0, 3).reshape(
                2, 128, KT * 256))
        # wo[h][d'][o] = Wo[o, (g*HG+h)*dh + d']
        wo_c = np.ascontiguousarray(Wo.T.reshape(H, DH, D)[hs])  # [HG, dh, D]
        in_maps.append({
            "xt": np.ascontiguousarray(x[b].T),
            "wq": wq_c, "wk": wk_c, "wv": wv_c, "wo": wo_c,
            "cosT": cosT[b], "sinT": sinT[b],
            "masks": masks, "permr": permr,
            "ones_c": ones_c, "ones_r": ones_r,
        })
    return in_maps


def kernel(x, positions, Wq, Wk, Wv, Wo, _profile=False):
    nc = _get_nc()
    in_maps = _host_prep(x, positions, Wq, Wk, Wv, Wo)
    res = run_bass_kernel_spmd(nc, in_maps, list(range(NCORES)), trace=_profile)
    out = np.zeros((B, L, D), np.float32)
    for core in range(NCORES):
        b = core // G
        out[b] += res.results[core]["out"]
    if _profile:
        kernel._last_exec_time_ns = res.exec_time_ns
        kernel._last_trace = res.instructions_and_trace
    return out



# revision 6
# speedup vs baseline: 1.7275x; 1.7275x over previous
"""Multi-head attention (RoPE, causal) Trainium2 kernel — v2.

Problem: B=2, L=2048, D=2048, H=16, dh=128, fp32.
Sharding: 8 cores = 2 batches x 4 head-groups (4 heads/core).
Each core computes QKV projections for its heads, RoPE, causal
attention, and a partial output projection (its heads' rows of Wo);
the host sums the 4 partials per batch.

v2 changes vs baseline:
 - all matmul operands fp16 (1 cyc/row like bf16, better mantissa,
   half the DMA, DVE 4x fast mode for fp16 SBUF-only elementwise ops)
 - weights SBUF-resident, DMA'd once (baseline re-loaded every chunk)
 - V projection emits [l, 4*dh] tiles via 16-step chains at ap=512
   (256 matmuls instead of 512 at ap=256)
 - softmax denominator: DVE-accumulated esum (fp16) + ONE ones-matmul
   per (head, chunk) instead of a PE matmul per k-tile (saves ~45us PE)
 - emission order software-pipelines PE work so the tensor engine
   never idles (idle gaps drop it to the 1.2GHz mid p-state)
 - Wo grouped for stationary reuse and interleaved into the next
   chunk's attention phase; out written fp16 (host upcasts+sums)
 - all DMAs issued from the Sync queue
"""
import sys
import numpy as np

sys.path.insert(0, '/opt/trn_rl_repo')

import concourse.bass as bass  # noqa: E402,F401
import concourse.mybir as mybir  # noqa: E402
import concourse.tile as tile  # noqa: E402
from concourse import bacc  # noqa: E402
from concourse import library_config  # noqa: E402
from concourse.bass_utils import run_bass_kernel_spmd  # noqa: E402

B, L, D = 2, 2048, 2048
H, DH = 16, 128
HG = 4           # heads per core
G = H // HG      # head groups (cores per batch)
NCORES = 8
CHUNK = 512      # l-chunk
NCH = L // CHUNK          # 4 chunks
KT = D // 128             # 16 k-tiles over D
LT = L // 128             # 16 l-tiles
ROPE_BASE = 10000.0

f32 = mybir.dt.float32
f16 = mybir.dt.float16

_built = None
PHASES = []


def _build():
    nc = bacc.Bacc()

    xt_d = nc.declare_dram_parameter("xt", [D, L], f16, isOutput=False)
    # wq/wk: [p][m*2048 + kt*128 + f] = W^T[kt*128+p, m*128+f] (deinterleaved
    # rows, scale folded into wq)
    wq_d = nc.declare_dram_parameter("wq", [128, HG * D], f16, isOutput=False)
    wk_d = nc.declare_dram_parameter("wk", [128, HG * D], f16, isOutput=False)
    # wv moving layout: [p][kt*512 + f] = Wv^T[kt*128+p, hs0*128 + f]
    wv_d = nc.declare_dram_parameter("wv", [128, KT * 512], f16, isOutput=False)
    # wo: [p][h*2048 + o] = Wo[o, (hs0+h)*128 + p]
    wo_d = nc.declare_dram_parameter("wo", [128, HG * D], f16, isOutput=False)
    cos_d = nc.declare_dram_parameter("cosT", [128, L], f16, isOutput=False)
    sin_d = nc.declare_dram_parameter("sinT", [128, L], f16, isOutput=False)
    tri_d = nc.declare_dram_parameter("tri", [128, 128], f16, isOutput=False)
    permr_d = nc.declare_dram_parameter("permr", [128, 128], f16, isOutput=False)
    ones_c_d = nc.declare_dram_parameter("ones_c", [128, 1], f16, isOutput=False)

    out_d = nc.declare_dram_parameter("out", [L, D], f16, isOutput=True)

    with tile.TileContext(nc) as tc:
        with (
            tc.tile_pool(name="const", bufs=1) as const,
            tc.tile_pool(name="persist", bufs=1) as persist,
            tc.tile_pool(name="xs", bufs=9) as xs,             # x-chunk quarters
            tc.tile_pool(name="qt", bufs=8) as qtp,            # qt per chunk
            tc.tile_pool(name="at", bufs=8) as atp,            # at per chunk
            tc.tile_pool(name="qraw", bufs=3) as qrawp,        # PSUM->SBUF f16
            tc.tile_pool(name="t12", bufs=4) as t12p,          # rope transients
            tc.tile_pool(name="et", bufs=6) as etp,            # exp tiles
            tc.tile_pool(name="es", bufs=3) as esp,            # esum tiles
            tc.tile_pool(name="sm", bufs=3) as smp,            # recip [1,512]
            tc.tile_pool(name="bc", bufs=2) as bcp,            # bcast [128,512]
            tc.tile_pool(name="ob", bufs=6) as obp,            # out staging
            tc.tile_pool(name="pacc", bufs=3, space="PSUM") as pacc,
            tc.tile_pool(name="pst", bufs=2, space="PSUM") as pst,
            tc.tile_pool(name="put", bufs=2, space="PSUM") as put,
            tc.tile_pool(name="prb", bufs=1, space="PSUM") as prb,
        ):
            # ---- constants / weights (one-time DMA) ----
            nc.gpsimd.load_library(library_config.attn)
            wq_t = const.tile([128, HG * D], f16)
            wk_t = const.tile([128, HG * D], f16)
            wv_t = const.tile([128, KT * 512], f16)
            wo_t = const.tile([128, HG * D], f16)
            cos_t = const.tile([128, L], f16)
            sin_t = const.tile([128, L], f16)
            tri_t = const.tile([128, 128], f16)
            permr_t = const.tile([128, 128], f16)
            ones_c = const.tile([128, 1], f16)
            nc.sync.dma_start(out=wq_t[:], in_=wq_d[:])
            nc.sync.dma_start(out=wk_t[:], in_=wk_d[:])
            nc.sync.dma_start(out=wv_t[:], in_=wv_d[:])
            nc.sync.dma_start(out=wo_t[:], in_=wo_d[:])
            nc.sync.dma_start(out=cos_t[:], in_=cos_d[:])
            nc.sync.dma_start(out=sin_t[:], in_=sin_d[:])
            nc.sync.dma_start(out=tri_t[:], in_=tri_d[:])
            nc.sync.dma_start(out=permr_t[:], in_=permr_d[:])
            nc.sync.dma_start(out=ones_c[:], in_=ones_c_d[:])

            # ---- persistent activations (full history) ----
            kt_t = [persist.tile([128, L], f16, name=f"ktt{h}") for h in range(HG)]
            v_t = [persist.tile([128, HG * 128], f16, name=f"vt{lt}")
                   for lt in range(LT)]

            def w_ap(wt, m, kt):
                return wt[:, m * D + kt * 128:m * D + kt * 128 + 128]

            at_tiles = {}      # (c, h) -> at tile
            qt_tiles = {}      # (c, h) -> qt tile

            def emit_wo_group(cc, sl):
                """Output projection for l-tile (cc*4+sl): 16 matmuls in two
                2-bank passes; at-slice stationary reused over 2 ot movings."""
                mt = cc * 4 + sl
                for otp in range(2):
                    wops = [pacc.tile([128, 512], f32, tag="acc",
                                      name=f"wop{mt}_{otp}_{oi}")
                            for oi in range(2)]
                    for h in range(HG):
                        a_sl = at_tiles[(cc, h)][:, sl * 128:(sl + 1) * 128]
                        for oi in range(2):
                            ot = otp * 2 + oi
                            nc.tensor.matmul(
                                wops[oi][:], a_sl,
                                wo_t[:, h * D + ot * 512:h * D + (ot + 1) * 512],
                                start=(h == 0), stop=(h == HG - 1))
                    for oi in range(2):
                        ot = otp * 2 + oi
                        osb = obp.tile([128, 512], f16, tag="ob")
                        nc.vector.tensor_copy(out=osb[:], in_=wops[oi][:])
                        nc.sync.dma_start(
                            out=out_d[mt * 128:(mt + 1) * 128,
                                      ot * 512:(ot + 1) * 512],
                            in_=osb[:])

            for c in range(NCH):
                PHASES.append((f"c{c}_load", int(nc.next_id())))
                cs = slice(c * CHUNK, (c + 1) * CHUNK)
                xc = []   # xc[q] = [128, 4, CHUNK]; k-tile kt -> xc[kt//4][:, kt%4, :]
                for q in range(4):
                    xq = xs.tile([128, 4, CHUNK], f16, tag="xc")
                    nc.sync.dma_start(
                        out=xq[:],
                        in_=xt_d[q * 512:(q + 1) * 512, cs].rearrange(
                            "(kt p) n -> p kt n", p=128))
                    xc.append(xq)

                def xtile(kt):
                    return xc[kt // 4][:, kt % 4, :]

                PHASES.append((f"c{c}_qk", int(nc.next_id())))
                # ---------- Q/K projections; RoPE pipelined one stage behind --
                pending = None   # (ps, dst_ap)

                def emit_rope(ps, dst_ap):
                    # dst = raw*cos + (R @ raw)*sin
                    qraw = qrawp.tile([128, CHUNK], f16, tag="qraw")
                    nc.scalar.copy(qraw[:], ps[:])
                    rot = pst.tile([128, CHUNK], f32, tag="st")
                    nc.tensor.matmul(rot[:], permr_t[:], qraw[:],
                                     start=True, stop=True)
                    t1 = t12p.tile([128, CHUNK], f16, tag="t12")
                    nc.vector.tensor_tensor(out=t1[:], in0=qraw[:],
                                            in1=cos_t[:, cs],
                                            op=mybir.AluOpType.mult)
                    t2 = t12p.tile([128, CHUNK], f16, tag="t12")
                    nc.vector.tensor_tensor(out=t2[:], in0=rot[:],
                                            in1=sin_t[:, cs],
                                            op=mybir.AluOpType.mult)
                    nc.vector.tensor_tensor(out=dst_ap, in0=t1[:], in1=t2[:],
                                            op=mybir.AluOpType.add)

                for (wt, isq) in ((wq_t, True), (wk_t, False)):
                    for m in range(HG):
                        ps = pacc.tile([128, CHUNK], f32, tag="acc")
                        for kt in range(KT):
                            nc.tensor.matmul(ps[:], w_ap(wt, m, kt), xtile(kt),
                                             start=(kt == 0), stop=(kt == KT - 1))
                        if pending is not None:
                            emit_rope(*pending)
                        if isq:
                            qt = qtp.tile([128, CHUNK], f16, tag="qt",
                                          name=f"qt_c{c}_h{m}")
                            qt_tiles[(c, m)] = qt
                            pending = (ps, qt[:])
                        else:
                            pending = (ps, kt_t[m][:, cs])

                PHASES.append((f"c{c}_v", int(nc.next_id())))
                # ---------- V projection: v_t[lt] = [l(128), 4 heads * 128] ----
                for sl in range(CHUNK // 128):
                    lt = c * (CHUNK // 128) + sl
                    vps = pacc.tile([128, 512], f32, tag="acc")
                    for kt in range(KT):
                        nc.tensor.matmul(
                            vps[:], xtile(kt)[:, sl * 128:(sl + 1) * 128],
                            wv_t[:, kt * 512:(kt + 1) * 512],
                            start=(kt == 0), stop=(kt == KT - 1))
                    if pending is not None:
                        emit_rope(*pending)
                        pending = None
                    nc.scalar.copy(v_t[lt][:], vps[:])

                PHASES.append((f"c{c}_attn", int(nc.next_id())))
                # ---------- attention for q-chunk c ----------
                nkt = (c + 1) * (CHUNK // 128)   # causal: k-tiles 0..nkt-1
                for h in range(HG):
                    qt = qt_tiles[(c, h)]
                    ut = put.tile([128, CHUNK], f32, tag="ut")
                    esum = esp.tile([128, CHUNK], f16, tag="es")
                    for kt in range(nkt):
                        # causal fine-grain: diag tile j only covers q >= 128j
                        diag_j = kt - c * 4
                        qs = (slice(diag_j * 128, CHUNK) if diag_j > 0
                              else slice(0, CHUNK))
                        st = pst.tile([128, CHUNK], f32, tag="st")
                        nc.tensor.matmul(st[:, qs],
                                         kt_t[h][:, kt * 128:(kt + 1) * 128],
                                         qt[:, qs], start=True, stop=True)
                        et = etp.tile([128, CHUNK], f16, tag="et")
                        nc.scalar.activation(et[:, qs], st[:, qs],
                                             mybir.ActivationFunctionType.Exp)
                        if diag_j >= 0:
                            js = slice(diag_j * 128, (diag_j + 1) * 128)
                            nc.vector.tensor_tensor(
                                out=et[:, js], in0=et[:, js], in1=tri_t[:],
                                op=mybir.AluOpType.mult)
                        nc.tensor.matmul(ut[:, qs],
                                         v_t[kt][:, h * 128:(h + 1) * 128],
                                         et[:, qs], start=(kt == 0),
                                         stop=(kt == nkt - 1),
                                         skip_group_check=True)
                        if kt == 0:
                            nc.vector.tensor_copy(out=esum[:], in_=et[:])
                        else:
                            nc.vector.tensor_tensor(out=esum[:, qs],
                                                    in0=esum[:, qs],
                                                    in1=et[:, qs],
                                                    op=mybir.AluOpType.add)
                    rs = prb.tile([1, CHUNK], f32, tag="rb")
                    nc.tensor.matmul(rs[:], ones_c[:], esum[:],
                                     start=True, stop=True)
                    recip = smp.tile([1, CHUNK], f32, tag="recip")
                    nc.vector.reciprocal_approx_fast(out=recip[:], in_=rs[:])
                    bc_sb = bcp.tile([128, CHUNK], f32, tag="bc")
                    nc.gpsimd.partition_broadcast(bc_sb[:], recip[:])
                    at = atp.tile([128, CHUNK], f16, tag="at",
                                  name=f"at_c{c}_h{h}")
                    at_tiles[(c, h)] = at
                    nc.vector.tensor_tensor(out=at[:], in0=ut[:],
                                            in1=bc_sb[:],
                                            op=mybir.AluOpType.mult)
                    # interleave previous chunk's output projection
                    if c >= 1:
                        emit_wo_group(c - 1, h)

                if c == NCH - 1:
                    PHASES.append((f"c{c}_wo", int(nc.next_id())))
                    for sl in range(4):
                        emit_wo_group(c, sl)

    nc.finalize()
    return nc


def _get_nc():
    global _built
    if _built is None:
        _built = _build()
    return _built


def _host_prep(x, positions, Wq, Wk, Wv, Wo):
    """Build per-core input maps."""
    x = np.asarray(x, np.float32)
    positions = np.asarray(positions)
    Wq = np.asarray(Wq, np.float32)
    Wk = np.asarray(Wk, np.float32)
    Wv = np.asarray(Wv, np.float32)
    Wo = np.asarray(Wo, np.float32)

    scale = np.float32(1.0 / np.sqrt(DH))
    perm = np.concatenate([np.arange(0, DH, 2), np.arange(1, DH, 2)])  # deinterleave

    Wq_p = (Wq * scale).reshape(H, DH, D)[:, perm, :]   # [H, dh, D]
    Wk_p = Wk.reshape(H, DH, D)[:, perm, :]

    # RoPE tables per batch (deinterleaved: first 64 = even dims, last 64 = odd)
    inv_freq = 1.0 / (ROPE_BASE ** (np.arange(0, DH, 2, dtype=np.float32) / DH))
    cosT = np.empty((B, 128, L), np.float32)
    sinT = np.empty((B, 128, L), np.float32)
    for b in range(B):
        freqs = positions[b].astype(np.float32)[:, None] * inv_freq[None, :]  # [L, 64]
        cb = np.cos(freqs).T.astype(np.float32)  # [64, L]
        sb = np.sin(freqs).T.astype(np.float32)
        cosT[b] = np.concatenate([cb, cb], axis=0)
        sinT[b] = np.concatenate([sb, sb], axis=0)

    # rotate-half signed permutation (in deinterleaved space), uploaded as R.T
    R = np.zeros((128, 128), np.float32)
    for i in range(64):
        R[i, i + 64] = -1.0
        R[i + 64, i] = 1.0
    permr = R.T.copy().astype(np.float16)

    # causal block mask (0/1, exact in fp16): tri[k, q] = k <= q
    tri = (np.arange(128)[:, None] <= np.arange(128)[None, :]).astype(np.float16)
    ones_c = np.ones((128, 1), np.float16)

    in_maps = []
    for core in range(NCORES):
        b, g = divmod(core, G)
        hs = slice(g * HG, (g + 1) * HG)
        # W^T for this core's heads: [D, HG*dh]
        wqT = Wq_p[hs].reshape(HG * DH, D).T          # [D, 512]
        wkT = Wk_p[hs].reshape(HG * DH, D).T
        wvT = Wv.reshape(H, DH, D)[hs].reshape(HG * DH, D).T
        # wq/wk: [p][m*2048 + kt*128 + f] = wT[kt*128+p, m*128+f]
        wq_c = np.ascontiguousarray(
            wqT.reshape(KT, 128, HG, DH).transpose(1, 2, 0, 3).reshape(
                128, HG * D)).astype(np.float16)
        wk_c = np.ascontiguousarray(
            wkT.reshape(KT, 128, HG, DH).transpose(1, 2, 0, 3).reshape(
                128, HG * D)).astype(np.float16)
        # wv: [p][kt*512 + f] = wvT[kt*128+p, f]
        wv_c = np.ascontiguousarray(
            wvT.reshape(KT, 128, 512).transpose(1, 0, 2).reshape(
                128, KT * 512)).astype(np.float16)
        # wo: [p][h*2048 + o] = Wo[o, (g*HG+h)*dh + p]
        wo_c = np.ascontiguousarray(
            Wo.T.reshape(H, DH, D)[hs].transpose(1, 0, 2).reshape(
                DH, HG * D)).astype(np.float16)
        in_maps.append({
            "xt": np.ascontiguousarray(x[b].T).astype(np.float16),
            "wq": wq_c, "wk": wk_c, "wv": wv_c, "wo": wo_c,
            "cosT": cosT[b].astype(np.float16),
            "sinT": sinT[b].astype(np.float16),
            "tri": tri, "permr": permr, "ones_c": ones_c,
        })
    return in_maps


def kernel(x, positions, Wq, Wk, Wv, Wo, _profile=False):
    nc = _get_nc()
    in_maps = _host_prep(x, positions, Wq, Wk, Wv, Wo)
    res = run_bass_kernel_spmd(nc, in_maps, list(range(NCORES)), trace=_profile)
    out = np.zeros((B, L, D), np.float32)
    for core in range(NCORES):
        b = core // G
        out[b] += res.results[core]["out"].astype(np.float32)
    if _profile:
        kernel._last_exec_time_ns = res.exec_time_ns
        kernel._last_trace = res.instructions_and_trace
    return out
